# revision 1
# baseline (speedup 1.0000x reference)
"""GCN block (DGL GraphConv norm='both' + ReLU) on 8 TRN2 NeuronCores.

Strategy (SPMD, one program for all cores; per-core data via inputs):
  - Nodes/edges sharded by destination: core c owns dst rows [c*6250, (c+1)*6250).
  - Every core builds the normalized feature table h = x * rsqrt(deg_out) in
    its own HBM (bf16, one 256B row per node), then uses SWDGE dma_gather to
    fetch h[src] for its edges (edge-major tiles of 128 rows). Gather calls
    round-robin the 4 SWDGE queues so descriptor generation runs on all four
    Q7 core pairs concurrently (the serial bottleneck otherwise).
  - Segment-sum by dst via TensorE: edges are sorted by (dst block of 256,
    src half), padded per (block, half) run to a uniform number of 128-edge
    tiles; per tile one matmul (lhsT = gathered rows [128e x 128f] bf16,
    rhs = is_equal one-hot [128e x 256d] bf16) accumulates the block's
    aggT[f, d] in PSUM.
  - Output: aggT columns x W (f32 matmul), scale rows by rsqrt(deg_in),
    + bias, ReLU, DMA out.

dma_gather indices are int16, so the table is split in two halves at row
32768; edges are grouped into two passes by source half.
"""

import sys

if "/opt/trn_rl_repo" not in sys.path:
    sys.path.insert(0, "/opt/trn_rl_repo")

import numpy as np
import ml_dtypes

import concourse.bacc as bacc
import concourse.mybir as mybir
from concourse.bass import AP
from concourse.bass_utils import run_bass_kernel_spmd
from concourse.tile import TileContext

N = 50000          # nodes
D = 128            # feature dim
NCORES = 8
NPC = N // NCORES  # 6250 dst nodes per core

R = 391            # table ranks per partition (128 * 391 = 50048 >= N)
RN = 128 * R       # padded node count = table rows
HALF = 32768       # int16 index limit; table split [0, HALF) / [HALF, RN)

DST_BLK = 256                     # dst nodes per PSUM block
NBLK = (NPC + DST_BLK - 1) // DST_BLK   # 25
D_PAD = NBLK * DST_BLK            # 6400
OCH = 49                          # output chunks of 128 dst rows (49*128=6272>=6250)

GCH_TILES = 16                    # gather chunk: 16 tiles = 2048 indices
RCH = 49                          # h-build chunk: ranks per DMA
NQ = 4                            # SWDGE queues used round-robin

F32 = mybir.dt.float32
BF16 = mybir.dt.bfloat16
I16 = mybir.dt.int16
I32 = mybir.dt.int32

TRACE = False            # set by test harness for profiling
LAST_RESULTS = None      # BassKernelResults of the last run


def _gather_idx_layout(vals):
    """[E] int16 -> [128, E//16] in dma_gather layout (16-wrap, 8x replicated)."""
    base = vals.reshape(-1, 16).T          # [16, E/16]
    return np.ascontiguousarray(np.tile(base, (8, 1)))


def _prep_inputs(x, edge_index, W, b):
    src = np.asarray(edge_index[0], dtype=np.int64)
    dst = np.asarray(edge_index[1], dtype=np.int64)
    E = src.shape[0]

    deg_out = np.bincount(src, minlength=N).astype(np.int32)
    deg_in = np.bincount(dst, minlength=N).astype(np.int32)

    core = dst // NPC
    half = (src >= HALF).astype(np.int64)
    dstl = dst - core * NPC                # local dst id
    blk = dstl // DST_BLK

    key = (core * NBLK + blk) * 2 + half   # (core, blk, half) group id
    counts = np.bincount(key, minlength=NCORES * NBLK * 2)
    T_BH = max(1, int(-(-counts.max() // 128)))  # uniform tiles per run
    RUN = T_BH * 128
    EPASS = NBLK * RUN                     # indices per pass per core
    TPASS = NBLK * T_BH                    # tiles per pass per core

    gstart = np.zeros(NCORES * NBLK * 2 + 1, dtype=np.int64)
    np.cumsum(counts, out=gstart[1:])
    order = np.lexsort((dstl, key))
    key_s = key[order]
    rank = np.arange(E, dtype=np.int64) - gstart[key_s]

    core_s = core[order]
    blk_s = blk[order]
    half_s = half[order]
    src_s = src[order]
    dloc_s = (dstl - blk * DST_BLK)[order]

    slot = (core_s * 2 + half_s) * EPASS + blk_s * RUN + rank

    idx_all = np.zeros(NCORES * 2 * EPASS, dtype=np.int16)  # dummy: row 0
    idx_all[slot] = np.where(half_s == 0, src_s, src_s - HALF).astype(np.int16)

    dloc_all = np.full(NCORES * 2 * EPASS, -1.0, dtype=np.float32)
    dloc_all[slot] = dloc_s
    dloc_all = dloc_all.astype(ml_dtypes.bfloat16)

    # replicated tensors
    xp = np.zeros((RN, D), dtype=ml_dtypes.bfloat16)
    xp[:N] = np.asarray(x, dtype=np.float32).astype(ml_dtypes.bfloat16)
    x_dev = np.ascontiguousarray(xp.reshape(128, R, D))

    dp = np.zeros(RN, dtype=np.int32)
    dp[:N] = deg_out
    degout_dev = np.ascontiguousarray(dp.reshape(128, R))

    W_dev = np.ascontiguousarray(np.asarray(W, dtype=np.float32))
    brep = np.ascontiguousarray(
        np.tile(np.asarray(b, dtype=np.float32)[None, :], (128, 1)))
    iota = np.ascontiguousarray(
        np.tile(np.arange(DST_BLK, dtype=np.float32)[None, :], (128, 1))
    ).astype(ml_dtypes.bfloat16)

    in_maps = []
    idx3 = idx_all.reshape(NCORES, 2, EPASS)
    dl3 = dloc_all.reshape(NCORES, 2, EPASS)
    for c in range(NCORES):
        di = np.zeros(OCH * 128, dtype=np.int32)
        di[:NPC] = deg_in[c * NPC:(c + 1) * NPC]
        degin_dev = np.ascontiguousarray(di.reshape(OCH, 128).T)
        in_maps.append({
            "x_dev": x_dev,
            "degout": degout_dev,
            "degin": degin_dev,
            "w": W_dev,
            "brep": brep,
            "iota": iota,
            "idx_a": _gather_idx_layout(idx3[c, 0]),
            "idx_b": _gather_idx_layout(idx3[c, 1]),
            # dloc per tile: [128, TPASS], tile t partition p = edge t*128+p
            "dloc_a": np.ascontiguousarray(dl3[c, 0].reshape(TPASS, 128).T),
            "dloc_b": np.ascontiguousarray(dl3[c, 1].reshape(TPASS, 128).T),
        })
    return in_maps, T_BH, EPASS, TPASS


def _build_program(T_BH):
    EPASS = NBLK * T_BH * 128
    TPASS = NBLK * T_BH

    nc = bacc.Bacc("TRN2", target_bir_lowering=False, debug=False,
                   num_devices=NCORES, num_swdge_queues=NQ)

    x_dev = nc.dram_tensor("x_dev", [128, R, D], BF16, kind="ExternalInput")
    degout = nc.dram_tensor("degout", [128, R], I32, kind="ExternalInput")
    degin = nc.dram_tensor("degin", [128, OCH], I32, kind="ExternalInput")
    w_d = nc.dram_tensor("w", [D, D], F32, kind="ExternalInput")
    brep_d = nc.dram_tensor("brep", [128, D], F32, kind="ExternalInput")
    iota_d = nc.dram_tensor("iota", [128, DST_BLK], BF16, kind="ExternalInput")
    idx_a = nc.dram_tensor("idx_a", [128, EPASS // 16], I16, kind="ExternalInput")
    idx_b = nc.dram_tensor("idx_b", [128, EPASS // 16], I16, kind="ExternalInput")
    dloc_a = nc.dram_tensor("dloc_a", [128, TPASS], BF16, kind="ExternalInput")
    dloc_b = nc.dram_tensor("dloc_b", [128, TPASS], BF16, kind="ExternalInput")
    y_d = nc.dram_tensor("y", [128, OCH, D], F32, kind="ExternalOutput")

    htab = nc.dram_tensor("htab", [RN, D], BF16)

    with TileContext(nc) as tc:
        with (
            tc.tile_pool(name="const", bufs=1) as cpool,
            tc.tile_pool(name="xstage", bufs=2) as xpool,
            tc.tile_pool(name="hstage", bufs=2) as hpool,
            tc.tile_pool(name="gbuf", bufs=4) as gpool,
            tc.tile_pool(name="ohbuf", bufs=4) as opool,
            tc.tile_pool(name="agg", bufs=1) as apool,
            tc.tile_pool(name="psum", bufs=6, space="PSUM") as ppool,
            tc.tile_pool(name="psum2", bufs=2, space="PSUM") as ppool2,
        ):
            # ---- constants / small loads ----
            idx_a_sb = cpool.tile([128, EPASS // 16], I16, tag="idxa")
            nc.sync.dma_start(out=idx_a_sb[:], in_=idx_a[:, :])
            idx_b_sb = cpool.tile([128, EPASS // 16], I16, tag="idxb")
            nc.sync.dma_start(out=idx_b_sb[:], in_=idx_b[:, :])
            dloc_a_sb = cpool.tile([128, TPASS], BF16, tag="dla")
            nc.sync.dma_start(out=dloc_a_sb[:], in_=dloc_a[:, :])
            dloc_b_sb = cpool.tile([128, TPASS], BF16, tag="dlb")
            nc.sync.dma_start(out=dloc_b_sb[:], in_=dloc_b[:, :])
            iota_sb = cpool.tile([128, DST_BLK], BF16, tag="iota")
            nc.sync.dma_start(out=iota_sb[:], in_=iota_d[:, :])
            w_sb = cpool.tile([D, D], F32, tag="w")
            nc.sync.dma_start(out=w_sb[:], in_=w_d[:, :])
            brep_sb = cpool.tile([128, D], F32, tag="brep")
            nc.sync.dma_start(out=brep_sb[:], in_=brep_d[:, :])

            # ---- norms ----
            degout_sb = cpool.tile([128, R], I32, tag="degout")
            nc.sync.dma_start(out=degout_sb[:], in_=degout[:, :])
            norm_src = cpool.tile([128, R], F32, tag="nsrc")
            nc.vector.tensor_copy(norm_src[:], degout_sb[:])
            nc.vector.tensor_scalar_max(norm_src[:], norm_src[:], 1.0)
            nc.vector.reciprocal(norm_src[:], norm_src[:])
            nc.scalar.activation(norm_src[:], norm_src[:],
                                 mybir.ActivationFunctionType.Sqrt)
            norm_src_bf = cpool.tile([128, R], BF16, tag="nsrcbf")
            nc.vector.tensor_copy(norm_src_bf[:], norm_src[:])

            degin_sb = cpool.tile([128, OCH], I32, tag="degin")
            nc.sync.dma_start(out=degin_sb[:], in_=degin[:, :])
            norm_dst = cpool.tile([128, OCH], F32, tag="ndst")
            nc.vector.tensor_copy(norm_dst[:], degin_sb[:])
            nc.vector.tensor_scalar_max(norm_dst[:], norm_dst[:], 1.0)
            nc.vector.reciprocal(norm_dst[:], norm_dst[:])
            nc.scalar.activation(norm_dst[:], norm_dst[:],
                                 mybir.ActivationFunctionType.Sqrt)

            # ---- h table build: h[n] = x[n] * norm_src[n], bf16 ----
            for r0 in range(0, R, RCH):
                rch = min(RCH, R - r0)
                xt = xpool.tile([128, RCH, D], BF16, tag="xt")
                nc.sync.dma_start(out=xt[:, :rch, :], in_=x_dev[:, r0:r0 + rch, :])
                ht = hpool.tile([128, RCH, D], BF16, tag="ht")
                nc.vector.tensor_tensor(
                    ht[:, :rch, :],
                    xt[:, :rch, :],
                    norm_src_bf[:, r0:r0 + rch, None].to_broadcast(
                        [128, rch, D]),
                    mybir.AluOpType.mult,
                )
                h_w = AP(htab, r0 * D, [[R * D, 128], [D, rch], [1, D]])
                nc.sync.dma_start(out=h_w, in_=ht[:, :rch, :])

            # gathers must not start before the full table is written
            tc.strict_bb_all_engine_barrier()

            # ---- aggregation passes ----
            aggT = apool.tile([128, D_PAD], F32, tag="aggT")
            h0 = htab[0:HALF, :]
            h1 = htab[HALF:RN, :]
            qn = 0
            for first, idx_sb, dl_sb, h_ap in (
                (True, idx_a_sb, dloc_a_sb, h0),
                (False, idx_b_sb, dloc_b_sb, h1),
            ):
                psum = None
                for t0 in range(0, TPASS, GCH_TILES):
                    nt = min(GCH_TILES, TPASS - t0)
                    nidx = nt * 128
                    g = gpool.tile([128, GCH_TILES, D], BF16, tag="g")
                    nc.gpsimd.dma_gather(
                        g[:, :nt, :],
                        h_ap,
                        idx_sb[:, t0 * 8:t0 * 8 + nidx // 16],
                        num_idxs=nidx,
                        num_idxs_reg=nidx,
                        elem_size=D,
                        single_packet=False,
                        queue_num=qn % NQ,
                    )
                    qn += 1
                    oh = opool.tile([128, GCH_TILES, DST_BLK], BF16, tag="oh")
                    nc.vector.tensor_tensor(
                        oh[:, :nt, :],
                        dl_sb[:, t0:t0 + nt, None].to_broadcast(
                            [128, nt, DST_BLK]),
                        iota_sb[:, None, :].to_broadcast([128, nt, DST_BLK]),
                        mybir.AluOpType.is_equal,
                    )
                    for tl in range(nt):
                        t = t0 + tl
                        blk = t // T_BH
                        k = t % T_BH
                        if k == 0:
                            psum = ppool.tile([128, DST_BLK], F32, tag="ps")
                        nc.tensor.matmul(
                            psum[:],
                            lhsT=g[:, tl, :],
                            rhs=oh[:, tl, :],
                            start=(k == 0),
                            stop=(k == T_BH - 1),
                        )
                        if k == T_BH - 1:
                            sl = aggT[:, blk * DST_BLK:(blk + 1) * DST_BLK]
                            if first:
                                nc.any.tensor_copy(sl, psum[:])
                            else:
                                nc.vector.tensor_add(sl, sl, psum[:])

            # ---- output: (aggT^T W) * norm_dst + b, ReLU ----
            outall = apool.tile([128, OCH, D], F32, tag="outall")
            for c in range(OCH):
                ps2 = ppool2.tile([128, D], F32, tag="ps2")
                nc.tensor.matmul(
                    ps2[:],
                    lhsT=aggT[:, c * 128:(c + 1) * 128],
                    rhs=w_sb[:],
                    start=True,
                    stop=True,
                )
                nc.vector.tensor_scalar(
                    outall[:, c, :], ps2[:], norm_dst[:, c:c + 1], None,
                    mybir.AluOpType.mult,
                )
            nc.vector.tensor_tensor(
                outall[:],
                outall[:],
                brep_sb[:, None, :].to_broadcast([128, OCH, D]),
                mybir.AluOpType.add,
            )
            nc.vector.tensor_scalar_max(outall[:], outall[:], 0.0)
            nc.sync.dma_start(out=y_d[:, :, :], in_=outall[:])

    nc.compile()
    return nc


def kernel(x, edge_index, W, b):
    global LAST_RESULTS
    x = np.asarray(x, dtype=np.float32)
    W = np.asarray(W, dtype=np.float32)
    b = np.asarray(b, dtype=np.float32)

    in_maps, T_BH, EPASS, TPASS = _prep_inputs(x, edge_index, W, b)
    nc = _build_program(T_BH)

    kwargs = {}
    if TRACE:
        kwargs["trace"] = True
    res = run_bass_kernel_spmd(nc, in_maps, list(range(NCORES)), **kwargs)
    LAST_RESULTS = res

    out = np.empty((N, D), dtype=np.float32)
    for c in range(NCORES):
        yc = np.asarray(res.results[c]["y"])          # [128, OCH, 128]
        rows = yc.transpose(1, 0, 2).reshape(OCH * 128, D)
        out[c * NPC:(c + 1) * NPC] = rows[:NPC]
    return out



# revision 3
# speedup vs baseline: 1.8211x; 1.8211x over previous
"""GCN block (DGL GraphConv norm='both' + ReLU) on 8 TRN2 NeuronCores.

Strategy (SPMD, one program for all cores; per-core data via inputs):
  - Nodes/edges sharded by destination: core c owns dst rows [c*6250, (c+1)*6250).
  - Host folds the source norm into the feature table: xh = x * rsqrt(deg_out)
    cast to bf16 (one 256B row per node). Each core gathers xh[src] for its
    edges straight from its HBM copy with SWDGE dma_gather — no on-device
    table build, no barrier; gathers start immediately.
  - Segment-sum by dst via TensorE: edges grouped by (128-wide dst block,
    src half) with a VARIABLE number of 128-edge tiles per group (max over
    cores, so the single SPMD program fits every core's data with ~2% padding
    instead of the ~32% a global max would cost). Per tile one matmul
    (lhsT = gathered rows [128e x 128f] bf16, rhs = is_equal one-hot
    [128e x 128d] bf16) accumulates the block's aggT[f, d] in PSUM; flushes
    add into a memset aggT so empty groups need no special casing.
  - Output stage is pipelined into pass B: as soon as block b's second-half
    flush lands, aggT_b x W (f32 matmul), scale by rsqrt(deg_in) (host-
    precomputed), + bias, ReLU, DMA out.

dma_gather indices are int16, so the table is addressed in two halves at row
32768; pass A covers half-0 edges, pass B half-1.
"""

import sys

if "/opt/trn_rl_repo" not in sys.path:
    sys.path.insert(0, "/opt/trn_rl_repo")

import numpy as np
import ml_dtypes

import concourse.bacc as bacc
import concourse.mybir as mybir
from concourse.bass_utils import run_bass_kernel_spmd
from concourse.tile import TileContext

N = 50000          # nodes
D = 128            # feature dim
NCORES = 8
NPC = N // NCORES  # 6250 dst nodes per core
RN = 50048         # padded table rows (multiple of 128)
HALF = 32768       # int16 index limit; table addressed [0, HALF) / [HALF, RN)

DST_BLK = 128                     # dst nodes per PSUM block
NBLK = (NPC + DST_BLK - 1) // DST_BLK   # 49
OCH = NBLK                        # output chunks of 128 dst rows

GCH = 32                          # gather chunk: tiles per dma_gather call
NQ = 4                            # SWDGE queues used round-robin

F32 = mybir.dt.float32
BF16 = mybir.dt.bfloat16
I16 = mybir.dt.int16
I32 = mybir.dt.int32

TRACE = False            # set by test harness for profiling
LAST_RESULTS = None      # BassKernelResults of the last run


def _gather_idx_layout(vals):
    """[S] int16 -> [128, S//16] in dma_gather layout (16-wrap, 8x replicated)."""
    base = vals.reshape(-1, 16).T          # [16, S/16]
    return np.ascontiguousarray(np.tile(base, (8, 1)))


def _prep_inputs(x, edge_index, W, b):
    src = np.asarray(edge_index[0], dtype=np.int64)
    dst = np.asarray(edge_index[1], dtype=np.int64)

    deg_out = np.bincount(src, minlength=N).astype(np.float32)
    deg_in = np.bincount(dst, minlength=N).astype(np.float32)
    nsrc = 1.0 / np.sqrt(np.maximum(deg_out, 1.0))
    ndst = 1.0 / np.sqrt(np.maximum(deg_in, 1.0))

    core = dst // NPC
    half = (src >= HALF).astype(np.int64)
    dstl = dst - core * NPC                # local dst id
    blk = dstl // DST_BLK                  # 0..NBLK-1

    # group = (half, blk); variable tiles per group = max over cores
    grp = half * NBLK + blk                # 0..2*NBLK-1
    key = core * (2 * NBLK) + grp
    counts = np.bincount(key, minlength=NCORES * 2 * NBLK).reshape(
        NCORES, 2 * NBLK)
    tiles_per_grp = -(-counts.max(axis=0) // 128)       # [2*NBLK] int
    grp_tile_start = np.zeros(2 * NBLK + 1, dtype=np.int64)
    np.cumsum(tiles_per_grp, out=grp_tile_start[1:])
    TTOT = int(grp_tile_start[-1])
    TA = int(tiles_per_grp[:NBLK].sum())   # tiles in pass A (half 0)

    # slot of each edge: per (core, grp) running rank
    gstart = np.zeros(NCORES * 2 * NBLK + 1, dtype=np.int64)
    np.cumsum(counts.reshape(-1), out=gstart[1:])
    order = np.argsort(key, kind="stable")
    rank = np.arange(len(src), dtype=np.int64) - gstart[key[order]]
    slot = core[order] * (TTOT * 128) + grp_tile_start[grp[order]] * 128 + rank

    idx_all = np.zeros(NCORES * TTOT * 128, dtype=np.int16)   # pad: row 0
    idx_all[slot] = (src[order] - half[order] * HALF).astype(np.int16)
    dloc_all = np.full(NCORES * TTOT * 128, -1.0, dtype=np.float32)
    dloc_all[slot] = (dstl - blk * DST_BLK)[order]
    dloc_all = dloc_all.astype(ml_dtypes.bfloat16)

    # replicated tensors
    xh = np.zeros((RN, D), dtype=ml_dtypes.bfloat16)
    xh[:N] = (np.asarray(x, dtype=np.float32) * nsrc[:, None]).astype(
        ml_dtypes.bfloat16)

    W_dev = np.ascontiguousarray(np.asarray(W, dtype=np.float32))
    brep = np.ascontiguousarray(
        np.tile(np.asarray(b, dtype=np.float32)[None, :], (128, 1)))
    iota = np.ascontiguousarray(
        np.tile(np.arange(DST_BLK, dtype=np.float32)[None, :], (128, 1))
    ).astype(ml_dtypes.bfloat16)

    in_maps = []
    idx3 = idx_all.reshape(NCORES, TTOT * 128)
    dl3 = dloc_all.reshape(NCORES, TTOT * 128)
    for c in range(NCORES):
        nd = np.ones(OCH * 128, dtype=np.float32)
        nd[:NPC] = ndst[c * NPC:(c + 1) * NPC]
        ndst_dev = np.ascontiguousarray(nd.reshape(OCH, 128).T)
        in_maps.append({
            "xh": xh,
            "w": W_dev,
            "brep": brep,
            "iota": iota,
            "ndst": ndst_dev,
            "idx": _gather_idx_layout(idx3[c]),
            # dloc per tile: [128, TTOT], tile t partition p = edge t*128+p
            "dloc": np.ascontiguousarray(dl3[c].reshape(TTOT, 128).T),
        })
    return in_maps, [int(t) for t in tiles_per_grp], TTOT, TA


def _build_program(tiles_per_grp, TTOT, TA):
    nc = bacc.Bacc("TRN2", target_bir_lowering=False, debug=False,
                   num_devices=NCORES, num_swdge_queues=NQ)

    xh_d = nc.dram_tensor("xh", [RN, D], BF16, kind="ExternalInput")
    w_d = nc.dram_tensor("w", [D, D], F32, kind="ExternalInput")
    brep_d = nc.dram_tensor("brep", [128, D], F32, kind="ExternalInput")
    iota_d = nc.dram_tensor("iota", [128, DST_BLK], BF16, kind="ExternalInput")
    ndst_d = nc.dram_tensor("ndst", [128, OCH], F32, kind="ExternalInput")
    idx_d = nc.dram_tensor("idx", [128, TTOT * 8], I16, kind="ExternalInput")
    dloc_d = nc.dram_tensor("dloc", [128, TTOT], BF16, kind="ExternalInput")
    y_d = nc.dram_tensor("y", [128, OCH, D], F32, kind="ExternalOutput")

    # per-tile metadata: (k within group, group size, blk, half)
    tmeta = []
    for g in range(2 * NBLK):
        T = tiles_per_grp[g]
        for k in range(T):
            tmeta.append((k, T, g % NBLK, g // NBLK))
    assert len(tmeta) == TTOT

    with TileContext(nc) as tc:
        with (
            tc.tile_pool(name="const", bufs=1) as cpool,
            tc.tile_pool(name="gbuf", bufs=4) as gpool,
            tc.tile_pool(name="ohbuf", bufs=4) as opool,
            tc.tile_pool(name="agg", bufs=1) as apool,
            tc.tile_pool(name="obuf", bufs=4) as obpool,
            tc.tile_pool(name="psum", bufs=6, space="PSUM") as ppool,
            tc.tile_pool(name="psum2", bufs=2, space="PSUM") as ppool2,
        ):
            # ---- constants / small loads ----
            idx_sb = cpool.tile([128, TTOT * 8], I16, tag="idx")
            nc.sync.dma_start(out=idx_sb[:], in_=idx_d[:, :])
            dloc_sb = cpool.tile([128, TTOT], BF16, tag="dl")
            nc.sync.dma_start(out=dloc_sb[:], in_=dloc_d[:, :])
            iota_sb = cpool.tile([128, DST_BLK], BF16, tag="iota")
            nc.sync.dma_start(out=iota_sb[:], in_=iota_d[:, :])
            w_sb = cpool.tile([D, D], F32, tag="w")
            nc.sync.dma_start(out=w_sb[:], in_=w_d[:, :])
            brep_sb = cpool.tile([128, D], F32, tag="brep")
            nc.sync.dma_start(out=brep_sb[:], in_=brep_d[:, :])
            ndst_sb = cpool.tile([128, OCH], F32, tag="ndst")
            nc.sync.dma_start(out=ndst_sb[:], in_=ndst_d[:, :])

            aggT = apool.tile([128, NBLK * DST_BLK], F32, tag="aggT")
            nc.vector.memset(aggT[:], 0.0)

            h0 = xh_d[0:HALF, :]
            h1 = xh_d[HALF:RN, :]

            def out_stage(blkid):
                ps2 = ppool2.tile([128, D], F32, tag="ps2")
                nc.tensor.matmul(
                    ps2[:],
                    lhsT=aggT[:, blkid * DST_BLK:(blkid + 1) * DST_BLK],
                    rhs=w_sb[:],
                    start=True,
                    stop=True,
                )
                ob = obpool.tile([128, D], F32, tag="ob")
                nc.vector.tensor_scalar(
                    ob[:], ps2[:], ndst_sb[:, blkid:blkid + 1], None,
                    mybir.AluOpType.mult,
                )
                nc.vector.tensor_add(ob[:], ob[:], brep_sb[:])
                nc.vector.tensor_scalar_max(ob[:], ob[:], 0.0)
                nc.sync.dma_start(out=y_d[:, blkid, :], in_=ob[:])

            qn = 0
            for base, npass, h_ap in ((0, TA, h0), (TA, TTOT - TA, h1)):
                psum = None
                for t0 in range(0, npass, GCH):
                    nt = min(GCH, npass - t0)
                    nidx = nt * 128
                    a0 = base + t0
                    g = gpool.tile([128, GCH, D], BF16, tag="g")
                    nc.gpsimd.dma_gather(
                        g[:, :nt, :],
                        h_ap,
                        idx_sb[:, a0 * 8:a0 * 8 + nidx // 16],
                        num_idxs=nidx,
                        num_idxs_reg=nidx,
                        elem_size=D,
                        single_packet=False,
                        queue_num=qn % NQ,
                    )
                    qn += 1
                    oh = opool.tile([128, GCH, DST_BLK], BF16, tag="oh")
                    nc.vector.tensor_tensor(
                        oh[:, :nt, :],
                        dloc_sb[:, a0:a0 + nt, None].to_broadcast(
                            [128, nt, DST_BLK]),
                        iota_sb[:, None, :].to_broadcast([128, nt, DST_BLK]),
                        mybir.AluOpType.is_equal,
                    )
                    for tl in range(nt):
                        k, T, blkid, halfid = tmeta[a0 + tl]
                        if k == 0:
                            psum = ppool.tile([128, DST_BLK], F32, tag="ps")
                        nc.tensor.matmul(
                            psum[:],
                            lhsT=g[:, tl, :],
                            rhs=oh[:, tl, :],
                            start=(k == 0),
                            stop=(k == T - 1),
                        )
                        if k == T - 1:
                            sl = aggT[:, blkid * DST_BLK:(blkid + 1) * DST_BLK]
                            nc.vector.tensor_add(sl, sl, psum[:])
                            if halfid == 1:
                                out_stage(blkid)

            # blocks with no half-1 tiles never got an out_stage above
            covered = set()
            for g_ in range(NBLK, 2 * NBLK):
                if tiles_per_grp[g_] > 0:
                    covered.add(g_ - NBLK)
            for blkid in range(NBLK):
                if blkid not in covered:
                    out_stage(blkid)

    nc.compile()
    return nc


def kernel(x, edge_index, W, b):
    global LAST_RESULTS
    x = np.asarray(x, dtype=np.float32)
    W = np.asarray(W, dtype=np.float32)
    b = np.asarray(b, dtype=np.float32)

    in_maps, tiles_per_grp, TTOT, TA = _prep_inputs(x, edge_index, W, b)
    nc = _build_program(tiles_per_grp, TTOT, TA)

    kwargs = {}
    if TRACE:
        kwargs["trace"] = True
    res = run_bass_kernel_spmd(nc, in_maps, list(range(NCORES)), **kwargs)
    LAST_RESULTS = res

    out = np.empty((N, D), dtype=np.float32)
    for c in range(NCORES):
        yc = np.asarray(res.results[c]["y"])          # [128, OCH, 128]
        rows = yc.transpose(1, 0, 2).reshape(OCH * 128, D)
        out[c * NPC:(c + 1) * NPC] = rows[:NPC]
    return out


# revision 4
# speedup vs baseline: 3.0144x; 1.6553x over previous
"""GCN block (DGL GraphConv norm='both' + ReLU) on 8 TRN2 NeuronCores.

Strategy (SPMD, one program for all cores; per-core data via inputs):
  - Nodes/edges sharded by destination: core c owns dst rows [c*6250, (c+1)*6250).
  - Host folds the source norm into the feature table: xh = x * rsqrt(deg_out)
    cast to bf16 (one 256B row per node). Each core gathers xh[src] for its
    edges straight from its HBM copy with SWDGE dma_gather — no on-device
    table build; gathers start immediately. SWDGE descriptor generation on
    the Q7 (~3.5-4.5ns/row, serial) is the kernel's critical path, so
    everything else is kept off it and overlapped under it.
  - Segment-sum by dst via TensorE: edges grouped by (128-wide dst block,
    src half) with a VARIABLE number of 128-edge tiles per group (max over
    cores, so the single SPMD program fits every core's data with ~6% padding
    instead of the ~32% a global max would cost). Per tile one matmul
    (lhsT = gathered rows [128e x 128f] bf16, rhs = one-hot [128e x 128d]
    bf16) accumulates the block's aggT[f, d] in PSUM; flushes add into a
    memset aggT so empty groups need no special casing.
  - The one-hot is built ON THE HOST with rsqrt(deg_in) pre-folded into its
    values and streamed in by DMA (27MB/core) — the vector engine does no
    one-hot work and no output scaling at all.
  - Output stage is pipelined into pass B: as soon as block b's second-half
    flush lands, aggT_b x W (f32 matmul), then ReLU on the scalar engine
    (bias is folded in only if b != 0), DMA out.

dma_gather indices are int16, so the table is addressed in two halves at row
32768; pass A covers half-0 edges, pass B half-1.
"""

import sys

if "/opt/trn_rl_repo" not in sys.path:
    sys.path.insert(0, "/opt/trn_rl_repo")

import numpy as np
import ml_dtypes

import concourse.bacc as bacc
import concourse.mybir as mybir
from concourse.bass_utils import run_bass_kernel_spmd
from concourse.tile import TileContext

N = 50000          # nodes
D = 128            # feature dim
NCORES = 8
NPC = N // NCORES  # 6250 dst nodes per core
RN = 50048         # padded table rows (multiple of 128)
HALF = 32768       # int16 index limit; table addressed [0, HALF) / [HALF, RN)

DST_BLK = 128                     # dst nodes per PSUM block
NBLK = (NPC + DST_BLK - 1) // DST_BLK   # 49
OCH = NBLK                        # output chunks of 128 dst rows

GCH = 32                          # gather chunk: tiles per dma_gather call
NQ = 4                            # SWDGE queues used round-robin

F32 = mybir.dt.float32
BF16 = mybir.dt.bfloat16
I16 = mybir.dt.int16

TRACE = False            # set by test harness for profiling
LAST_RESULTS = None      # BassKernelResults of the last run


def _gather_idx_layout(vals):
    """[S] int16 -> [128, S//16] in dma_gather layout (16-wrap, 8x replicated)."""
    base = vals.reshape(-1, 16).T          # [16, S/16]
    return np.ascontiguousarray(np.tile(base, (8, 1)))


def _prep_inputs(x, edge_index, W, b):
    src = np.asarray(edge_index[0], dtype=np.int64)
    dst = np.asarray(edge_index[1], dtype=np.int64)

    deg_out = np.bincount(src, minlength=N).astype(np.float32)
    deg_in = np.bincount(dst, minlength=N).astype(np.float32)
    nsrc = 1.0 / np.sqrt(np.maximum(deg_out, 1.0))
    ndst = 1.0 / np.sqrt(np.maximum(deg_in, 1.0))

    core = dst // NPC
    half = (src >= HALF).astype(np.int64)
    dstl = dst - core * NPC                # local dst id
    blk = dstl // DST_BLK                  # 0..NBLK-1

    # group = (half, blk); variable tiles per group = max over cores
    grp = half * NBLK + blk                # 0..2*NBLK-1
    key = core * (2 * NBLK) + grp
    counts = np.bincount(key, minlength=NCORES * 2 * NBLK).reshape(
        NCORES, 2 * NBLK)
    tiles_per_grp = -(-counts.max(axis=0) // 128)       # [2*NBLK] int
    grp_tile_start = np.zeros(2 * NBLK + 1, dtype=np.int64)
    np.cumsum(tiles_per_grp, out=grp_tile_start[1:])
    TTOT = int(grp_tile_start[-1])
    TA = int(tiles_per_grp[:NBLK].sum())   # tiles in pass A (half 0)

    # slot of each edge: per (core, grp) running rank
    gstart = np.zeros(NCORES * 2 * NBLK + 1, dtype=np.int64)
    np.cumsum(counts.reshape(-1), out=gstart[1:])
    order = np.argsort(key, kind="stable")
    rank = np.arange(len(src), dtype=np.int64) - gstart[key[order]]
    slot = core[order] * (TTOT * 128) + grp_tile_start[grp[order]] * 128 + rank

    idx_all = np.zeros(NCORES * TTOT * 128, dtype=np.int16)   # pad: row 0
    idx_all[slot] = (src[order] - half[order] * HALF).astype(np.int16)

    # host one-hot with ndst folded in: oh[slot, d] = ndst[dst] at d = dloc
    oh_all = np.zeros((NCORES * TTOT * 128, DST_BLK), dtype=ml_dtypes.bfloat16)
    dloc = (dstl - blk * DST_BLK)[order]
    oh_all[slot, dloc] = ndst[dst[order]].astype(ml_dtypes.bfloat16)

    # replicated tensors
    xh = np.zeros((RN, D), dtype=ml_dtypes.bfloat16)
    xh[:N] = (np.asarray(x, dtype=np.float32) * nsrc[:, None]).astype(
        ml_dtypes.bfloat16)

    W_dev = np.ascontiguousarray(np.asarray(W, dtype=np.float32))
    use_bias = bool(np.any(np.asarray(b, dtype=np.float32) != 0.0))
    brep = np.ascontiguousarray(
        np.tile(np.asarray(b, dtype=np.float32)[None, :], (128, 1)))

    in_maps = []
    idx3 = idx_all.reshape(NCORES, TTOT * 128)
    oh4 = oh_all.reshape(NCORES, TTOT, 128, DST_BLK)
    for c in range(NCORES):
        in_maps.append({
            "xh": xh,
            "w": W_dev,
            "brep": brep,
            "idx": _gather_idx_layout(idx3[c]),
            # oh tile t lane e col d -> oh_dev[e, t, d]
            "oh": np.ascontiguousarray(oh4[c].transpose(1, 0, 2)),
        })
    return in_maps, [int(t) for t in tiles_per_grp], TTOT, TA, use_bias


def _build_program(tiles_per_grp, TTOT, TA, use_bias):
    nc = bacc.Bacc("TRN2", target_bir_lowering=False, debug=False,
                   num_devices=NCORES, num_swdge_queues=NQ)

    xh_d = nc.dram_tensor("xh", [RN, D], BF16, kind="ExternalInput")
    w_d = nc.dram_tensor("w", [D, D], F32, kind="ExternalInput")
    brep_d = nc.dram_tensor("brep", [128, D], F32, kind="ExternalInput")
    idx_d = nc.dram_tensor("idx", [128, TTOT * 8], I16, kind="ExternalInput")
    oh_d = nc.dram_tensor("oh", [128, TTOT, DST_BLK], BF16,
                          kind="ExternalInput")
    y_d = nc.dram_tensor("y", [128, OCH, D], F32, kind="ExternalOutput")

    # per-tile metadata: (k within group, group size, blk, half)
    tmeta = []
    for g in range(2 * NBLK):
        T = tiles_per_grp[g]
        for k in range(T):
            tmeta.append((k, T, g % NBLK, g // NBLK))
    assert len(tmeta) == TTOT

    with TileContext(nc) as tc:
        with (
            tc.tile_pool(name="const", bufs=1) as cpool,
            tc.tile_pool(name="gbuf", bufs=6) as gpool,
            tc.tile_pool(name="ohbuf", bufs=6) as opool,
            tc.tile_pool(name="agg", bufs=1) as apool,
            tc.tile_pool(name="obuf", bufs=4) as obpool,
            tc.tile_pool(name="psum", bufs=6, space="PSUM") as ppool,
            tc.tile_pool(name="psum2", bufs=2, space="PSUM") as ppool2,
        ):
            # ---- constants / small loads ----
            idx_sb = cpool.tile([128, TTOT * 8], I16, tag="idx")
            nc.sync.dma_start(out=idx_sb[:], in_=idx_d[:, :])
            w_sb = cpool.tile([D, D], F32, tag="w")
            nc.sync.dma_start(out=w_sb[:], in_=w_d[:, :])
            brep_sb = cpool.tile([128, D], F32, tag="brep")
            nc.sync.dma_start(out=brep_sb[:], in_=brep_d[:, :])

            aggT = apool.tile([128, NBLK * DST_BLK], F32, tag="aggT")
            nc.vector.memset(aggT[:], 0.0)

            h0 = xh_d[0:HALF, :]
            h1 = xh_d[HALF:RN, :]

            def out_stage(blkid):
                ps2 = ppool2.tile([128, D], F32, tag="ps2")
                nc.tensor.matmul(
                    ps2[:],
                    lhsT=aggT[:, blkid * DST_BLK:(blkid + 1) * DST_BLK],
                    rhs=w_sb[:],
                    start=True,
                    stop=True,
                )
                ob = obpool.tile([128, D], F32, tag="ob")
                if use_bias:
                    nc.vector.tensor_add(ob[:], ps2[:], brep_sb[:])
                    nc.vector.tensor_scalar_max(ob[:], ob[:], 0.0)
                else:
                    nc.scalar.activation(ob[:], ps2[:],
                                         mybir.ActivationFunctionType.Relu)
                nc.sync.dma_start(out=y_d[:, blkid, :], in_=ob[:])

            qn = 0
            for base, npass, h_ap in ((0, TA, h0), (TA, TTOT - TA, h1)):
                psum = None
                for t0 in range(0, npass, GCH):
                    nt = min(GCH, npass - t0)
                    nidx = nt * 128
                    a0 = base + t0
                    g = gpool.tile([128, GCH, D], BF16, tag="g")
                    nc.gpsimd.dma_gather(
                        g[:, :nt, :],
                        h_ap,
                        idx_sb[:, a0 * 8:a0 * 8 + nidx // 16],
                        num_idxs=nidx,
                        num_idxs_reg=nidx,
                        elem_size=D,
                        single_packet=False,
                        queue_num=qn % NQ,
                    )
                    qn += 1
                    oh = opool.tile([128, GCH, DST_BLK], BF16, tag="oh")
                    nc.sync.dma_start(out=oh[:, :nt, :],
                                      in_=oh_d[:, a0:a0 + nt, :])
                    for tl in range(nt):
                        k, T, blkid, halfid = tmeta[a0 + tl]
                        if k == 0:
                            psum = ppool.tile([128, DST_BLK], F32, tag="ps")
                        nc.tensor.matmul(
                            psum[:],
                            lhsT=g[:, tl, :],
                            rhs=oh[:, tl, :],
                            start=(k == 0),
                            stop=(k == T - 1),
                        )
                        if k == T - 1:
                            sl = aggT[:, blkid * DST_BLK:(blkid + 1) * DST_BLK]
                            nc.vector.tensor_add(sl, sl, psum[:])
                            if halfid == 1:
                                out_stage(blkid)

            # blocks with no half-1 tiles never got an out_stage above
            for blkid in range(NBLK):
                if tiles_per_grp[NBLK + blkid] == 0:
                    out_stage(blkid)

    nc.compile()
    return nc


def kernel(x, edge_index, W, b):
    global LAST_RESULTS
    x = np.asarray(x, dtype=np.float32)
    W = np.asarray(W, dtype=np.float32)
    b = np.asarray(b, dtype=np.float32)

    in_maps, tiles_per_grp, TTOT, TA, use_bias = _prep_inputs(
        x, edge_index, W, b)
    nc = _build_program(tiles_per_grp, TTOT, TA, use_bias)

    kwargs = {}
    if TRACE:
        kwargs["trace"] = True
    res = run_bass_kernel_spmd(nc, in_maps, list(range(NCORES)), **kwargs)
    LAST_RESULTS = res

    out = np.empty((N, D), dtype=np.float32)
    for c in range(NCORES):
        yc = np.asarray(res.results[c]["y"])          # [128, OCH, 128]
        rows = yc.transpose(1, 0, 2).reshape(OCH * 128, D)
        out[c * NPC:(c + 1) * NPC] = rows[:NPC]
    return out
